# revision 5
# baseline (speedup 1.0000x reference)
"""Distributed Trainium2 kernel for a single causal attention head.

Problem (hardcoded): B=4, S=2048, D_MODEL=1024, HEAD_DIM=64, fp32 inputs.
    q = query @ Wq + bq ; k = key @ Wk + bk ; v = value @ Wv + bv
    scores = q k^T / sqrt(H) ; masked softmax ; out = att @ v

Sharding (8 NeuronCores): core c = (b, h) with b = c//2, h = c%2.
Each core owns 4 query chunks of 256 rows of batch b and projects the
full k/v of its batch locally.  Chunks h=0 -> {0,3,4,7}, h=1 ->
{1,2,5,6} balance causal work under one SPMD program.  The program
works in two slot-PAIRS of 512 query rows; pair p computes j-tiles
[0, SHARED_p) at 512 width and [SHARED_p, SOLO_p) at 256 width (second
chunk only).  Causal: SHARED=(4,12), SOLO=(8,16) -- optimal given both
cores' chunk extents.  Per-core differences are pure data: gathered
query rows and device-generated predicate masks (from threshold
tables) that zero attention weights after exp.

v2 performance notes (vs v1 baseline, 60.7us):
  - All big inputs ride ONE HWDGE sync ring with 16KB-contiguous
    per-partition lines (kT/vT packed half-major [P,2,DCH,S/2]; qT one
    whole transfer) -> max descriptor size, ~358 GB/s instead of 313.
  - Ring order k0, q, k1, v0, v1: scores (k x q) start earliest.
  - ACT engine runs ONLY the 20 exp tiles (the mid-kernel pacer at
    ~680ns/512-wide tile).  Projection PSUM drains (+bias, fp32->bf16)
    moved to DVE tensor_scalar; predicate muls moved to GpSimd.
  - q projection col-paired like k/v (4096 PE cycles, was 8192).
  - Solo score tiles exp'd in PAIRS sharing one PSUM bank (one 512-wide
    exp for two 256-wide tiles) -> 20 exps instead of 24.
  - Output staged [P, NQ/P, H] and DMA'd packed (2KB contiguous per
    partition); host unpacks the (t p) -> rows permutation.
  - PE warm-up/bridge matmuls sized to keep HAM at 2.4 GHz through
    every data-wait gap (no >3us PE idle).
"""

import os

import numpy as np
import ml_dtypes

import concourse.bass as bass
import concourse.tile as tile
from concourse import bacc, mybir
from concourse.bass import ds
from concourse.bass_utils import run_bass_kernel_spmd
from concourse.masks import make_identity

B, S, D, H = 4, 2048, 1024, 64
P = 128
NCORES = 8
CHUNK = 256               # query rows per chunk/slot
NSLOTS = 4
NQ = NSLOTS * CHUNK       # 1024
JT = S // P               # 16 j-tiles of 128 keys
DCH = D // P              # 8 contraction chunks
HS = S // 2               # 1024 cols per half transfer
FP = mybir.dt.float32
BF = mybir.dt.bfloat16
BF_NP = ml_dtypes.bfloat16

# (shared extent, solo extent) per pair, in j-tiles
CAUSAL_PAIRS = ((4, 8), (12, 16))
CAUSAL_CHUNKS = {0: (0, 3, 4, 7), 1: (1, 2, 5, 6)}
# predicate entries: (pair, jt) for jt in [8p, 8p+8)
CAUSAL_MASKED = [(p, jt) for p in range(2) for jt in range(8 * p, 8 * p + 8)]

# warm-up / bridge matmul counts (N=512 each, ~216ns warm / ~427 cold)
WARM_A = 22       # kernel start -> k0 arrival (~6.3us to cover)
BRIDGE_KQ = 13    # kproj0 end -> q arrival (~3.9us)
BRIDGE_V1 = 6     # epi0 -> v1 arrival

LAST_RESULTS = None
_PROGRAM_CACHE = {}


def _build_program(pairs, masked_slots):
    """Build the SPMD Bass program (identical on all 8 cores)."""
    nc = bacc.Bacc("TRN2", target_bir_lowering=False, debug=False,
                   num_devices=NCORES)

    qT_d = nc.dram_tensor("qT", [P, DCH, NQ], BF, kind="ExternalInput").ap()
    kT_d = nc.dram_tensor("kT", [P, 2, DCH, HS], BF,
                          kind="ExternalInput").ap()
    vT_d = nc.dram_tensor("vT", [P, 2, DCH, HS], BF,
                          kind="ExternalInput").ap()
    wall_d = nc.dram_tensor("wall", [P, DCH, 3 * H], BF,
                            kind="ExternalInput").ap()
    ball_d = nc.dram_tensor("ball", [H, 3], FP, kind="ExternalInput").ap()
    nmask = len(masked_slots)
    thr_d = nc.dram_tensor("thr", [P, nmask, 2], FP,
                           kind="ExternalInput").ap()
    out_d = nc.dram_tensor("out", [P, NQ // P, H], FP,
                           kind="ExternalOutput").ap()

    with tile.TileContext(nc) as tc:
        with (
            tc.tile_pool(name="const", bufs=1) as const,
            tc.tile_pool(name="resident", bufs=1) as res,
            tc.tile_pool(name="attp", bufs=26) as attp,
            tc.tile_pool(name="outp", bufs=2) as outp,
            tc.tile_pool(name="pp", bufs=1, space="PSUM") as pp,
            tc.tile_pool(name="psc", bufs=3, space="PSUM") as psc,
            tc.tile_pool(name="pout", bufs=2, space="PSUM") as pout,
            tc.tile_pool(name="ptr", bufs=2, space="PSUM") as ptr,
        ):
            # ---- constants on the scalar HWDGE ring ----
            wall_sb = const.tile([P, DCH, 3 * H], BF, tag="wall")
            nc.scalar.dma_start(wall_sb, wall_d)
            ball_sb = const.tile([H, 3], FP, tag="ball")
            nc.scalar.dma_start(ball_sb, ball_d)
            thr_sb = const.tile([P, nmask, 2], FP, tag="thr")
            nc.scalar.dma_start(thr_sb, thr_d)
            wk_sb = wall_sb[:, :, 0:H]
            wv_sb = wall_sb[:, :, H:2 * H]
            wq_sb = wall_sb[:, :, 2 * H:3 * H]
            bk_sb = ball_sb[:, 0:1]
            bv_sb = ball_sb[:, 1:2]
            bq_sb = ball_sb[:, 2:3]
            zeros_sb = const.tile([P, 512], BF, tag="zeros")
            nc.vector.memset(zeros_sb, 0.0)
            ident = const.tile([P, P], FP, tag="ident")
            make_identity(nc, ident)
            identb = const.tile([P, P], BF, tag="identb")
            make_identity(nc, identb)

            # ---- big inputs: ONE sync HWDGE ring, 16KB lines, strict
            # dependency order k0, q, k1, v0, v1 ----
            xk_sb = res.tile([P, 2, DCH, HS], BF, tag="xk")
            xv_sb = res.tile([P, 2, DCH, HS], BF, tag="xv")
            xq_sb = res.tile([P, DCH, NQ], BF, tag="xq")
            nc.sync.dma_start(xk_sb[:, 0], kT_d[:, 0])
            nc.sync.dma_start(xq_sb, qT_d)
            nc.sync.dma_start(xk_sb[:, 1], kT_d[:, 1])
            nc.sync.dma_start(xv_sb[:, 0], vT_d[:, 0])
            nc.sync.dma_start(xv_sb[:, 1], vT_d[:, 1])

            # predicate masks from per-core threshold tables:
            # pred[p, mi, half*256+f] = (f >= thr[p, mi, half])
            pred_sb = res.tile([P, nmask, 2 * CHUNK], BF, tag="pred")
            iota_sb = const.tile([P, CHUNK], FP, tag="iota")
            nc.gpsimd.iota(iota_sb, pattern=[[1, CHUNK]], base=0,
                           channel_multiplier=0,
                           allow_small_or_imprecise_dtypes=True)
            for mi in range(nmask):
                for half in range(2):
                    nc.gpsimd.tensor_scalar(
                        pred_sb[:, mi, ds(half * CHUNK, CHUNK)],
                        iota_sb, thr_sb[:, mi, ds(half, 1)], None,
                        mybir.AluOpType.is_ge)

            # ---- PE warm-up: keep HAM at full clock until k0 lands
            pwarm = pp.tile([P, 512], FP, tag="pp", name="pwarm")

            def bridge(n):
                for _ in range(n):
                    nc.tensor.matmul(pwarm, lhsT=identb, rhs=zeros_sb,
                                     start=True, stop=True)

            bridge(WARM_A)

            # col-tiled projection pair: two M=64 matmuls run concurrently
            # in PE column groups 0 / 1 over two 512-wide input chunks.
            # x3: [P, DCH, 1024]; gc0: global output column of x3[..., 0]
            def proj_pair(w_sb, x3, gc0, out_fn, name):
                pj = pp.tile([P, 512], FP, tag="pp", name=name)
                for d in range(DCH):
                    nc.tensor.matmul(pj[0:H, :], lhsT=w_sb[:, d, :],
                                     rhs=x3[:, d, ds(0, 512)],
                                     start=(d == 0), stop=(d == DCH - 1),
                                     skip_group_check=True)
                    nc.tensor.matmul(pj[H:2 * H, :], lhsT=w_sb[:, d, :],
                                     rhs=x3[:, d, ds(512, 512)],
                                     start=(d == 0), stop=(d == DCH - 1),
                                     tile_position=(0, H),
                                     skip_group_check=True)
                out_fn(pj[0:H, :], gc0)
                out_fn(pj[H:2 * H, :], gc0 + 512)

            k_sb = res.tile([P, S], BF, tag="k")
            vT_sb = res.tile([P, S], BF, tag="vT")
            v_sb = res.tile([P, JT, H + 1], BF, tag="v")
            nc.vector.memset(v_sb[:, :, H:H + 1], 1.0)
            q_sb = res.tile([P, NQ], BF, tag="q")
            # zero q rows 64.. so the K=128 score contraction ignores
            # garbage k rows 64.. (zeroing one operand suffices)
            nc.vector.memset(q_sb[H:, :], 0.0)

            # PSUM drains on DVE (ACT is reserved for exp)
            def k_out(pj, c0):
                nc.vector.tensor_scalar(k_sb[:H, ds(c0, 512)], pj,
                                        bk_sb, None, mybir.AluOpType.add)

            def v_out(pj, c0):
                nc.vector.tensor_scalar(vT_sb[:H, ds(c0, 512)], pj,
                                        bv_sb, None, mybir.AluOpType.add)
                for jt in range(c0 // P, c0 // P + 4):
                    pvt = ptr.tile([P, P], BF, tag="tr", name="pvt")
                    nc.tensor.transpose(pvt, vT_sb[:, ds(jt * P, P)], identb)
                    nc.vector.tensor_copy(v_sb[:, jt, 0:H], pvt[:, :H])

            def q_out(pj, c0):
                nc.vector.tensor_scalar(q_sb[:H, ds(c0, 512)], pj,
                                        bq_sb, None, mybir.AluOpType.add)

            mask_idx = {sj: i for i, sj in enumerate(masked_slots)}
            W = 2 * CHUNK  # 512
            po_tiles = {}
            att_tiles = {}

            def emit_score_wide(pr, jt):
                c0 = pr * W
                ps = psc.tile([P, W], FP, tag="sc", name="ps")
                nc.tensor.matmul(ps, lhsT=k_sb[:, ds(jt * P, P)],
                                 rhs=q_sb[:, ds(c0, W)],
                                 start=True, stop=True)
                att = attp.tile([P, W], BF, tag="att", name="att")
                nc.scalar.activation(att, ps,
                                     mybir.ActivationFunctionType.Exp,
                                     scale=0.125)
                mi = mask_idx.get((pr, jt))
                if mi is not None:
                    nc.gpsimd.tensor_mul(att, att, pred_sb[:, mi, :])
                att_tiles[(pr, jt)] = (att, c0, W)

            def emit_score_solo_pair(pr, jt0):
                # two 256-wide solo tiles (jt0, jt0+1) share one PSUM
                # bank and one 512-wide exp
                c0 = pr * W + CHUNK
                ps = psc.tile([P, W], FP, tag="sc", name="ps")
                for i in range(2):
                    nc.tensor.matmul(ps[:, ds(i * CHUNK, CHUNK)],
                                     lhsT=k_sb[:, ds((jt0 + i) * P, P)],
                                     rhs=q_sb[:, ds(c0, CHUNK)],
                                     start=True, stop=True,
                                     skip_group_check=True)
                att = attp.tile([P, W], BF, tag="att", name="att")
                nc.scalar.activation(att, ps,
                                     mybir.ActivationFunctionType.Exp,
                                     scale=0.125)
                for i in range(2):
                    mi = mask_idx.get((pr, jt0 + i))
                    if mi is not None:
                        nc.gpsimd.tensor_mul(
                            att[:, ds(i * CHUNK, CHUNK)],
                            att[:, ds(i * CHUNK, CHUNK)],
                            pred_sb[:, mi, ds(CHUNK, CHUNK)])
                    att_tiles[(pr, jt0 + i)] = (att, c0, CHUNK, i * CHUNK)

            def emit_av(pr, jt):
                solo = pairs[pr][1]
                if pr not in po_tiles:
                    po_tiles[pr] = pout.tile([H + 1, W], FP, tag="po",
                                             name=f"po{pr}")
                rec = att_tiles.pop((pr, jt))
                att, c0, n = rec[0], rec[1], rec[2]
                src = att if len(rec) == 3 else att[:, ds(rec[3], n)]
                nc.tensor.matmul(po_tiles[pr][:, ds(c0 - pr * W, n)],
                                 lhsT=v_sb[:, jt, :], rhs=src,
                                 start=(jt == 0), stop=(jt == solo - 1),
                                 skip_group_check=True)

            out_stage = res.tile([P, NQ // P, H], FP, tag="ostage")

            def epilogue(pr):
                po = po_tiles[pr]
                oT_sb = outp.tile([P, W], FP, tag="oT")
                nc.vector.tensor_copy(oT_sb[:H + 1, :], po)
                for t in range(W // P):
                    pt = ptr.tile([P, P], FP, tag="tr")
                    nc.tensor.transpose(pt, oT_sb[:, ds(t * P, P)], ident)
                    recip = outp.tile([P, 1], FP, tag="recip")
                    nc.vector.reciprocal(recip, pt[:, H:H + 1])
                    nc.vector.tensor_scalar_mul(
                        out_stage[:, pr * (W // P) + t, :], pt[:, :H], recip)

            # ---- decoupled emission schedule (engines are in-order;
            # each group sits at its data-arrival position) ----
            sh0, so0 = pairs[0]
            sh1, so1 = pairs[1]

            proj_pair(wk_sb, xk_sb[:, 0], 0, k_out, "pk0")
            bridge(BRIDGE_KQ)
            proj_pair(wq_sb, xq_sb, 0, q_out, "pq")
            for jt in range(sh0):
                emit_score_wide(0, jt)
            for jt0 in range(sh0, so0, 2):
                emit_score_solo_pair(0, jt0)
            for jt in range(8):
                emit_score_wide(1, jt)
            proj_pair(wk_sb, xk_sb[:, 1], HS, k_out, "pk1")
            for jt in range(8, sh1):
                emit_score_wide(1, jt)
            for jt0 in range(sh1, so1, 2):
                emit_score_solo_pair(1, jt0)
            proj_pair(wv_sb, xv_sb[:, 0], 0, v_out, "pv0")
            for jt in range(so0):
                emit_av(0, jt)
            for jt in range(8):
                emit_av(1, jt)
            epilogue(0)
            bridge(BRIDGE_V1)
            proj_pair(wv_sb, xv_sb[:, 1], HS, v_out, "pv1")
            for jt in range(8, so1):
                emit_av(1, jt)
            epilogue(1)
            nc.gpsimd.dma_start(out_d, out_stage)

    nc.compile()
    return nc


def _slot_extents(pairs):
    return (pairs[0][0], pairs[0][1], pairs[1][0], pairs[1][1])


def _mask_fits_causal_variant(mask):
    extents = _slot_extents(CAUSAL_PAIRS)
    for h, chunks in CAUSAL_CHUNKS.items():
        for s, g in enumerate(chunks):
            rows = slice(g * CHUNK, (g + 1) * CHUNK)
            bound = extents[s] * P
            lo = (8 * (s // 2)) * P
            if bound < S and mask[:, rows, bound:].any():
                return False
            if lo > 0 and not mask[:, rows, :lo].all():
                return False
    return True


def _pack_q(xT):
    """[D, NQ] -> [128, D/128, NQ]."""
    d, s = xT.shape
    return np.ascontiguousarray(
        xT.reshape(DCH, P, s).transpose(1, 0, 2)).astype(BF_NP)


def _pack_kv(xT):
    """[D, S] -> [128, 2, D/128, S/2]: 16KB contiguous per partition
    per half transfer."""
    d, s = xT.shape
    return np.ascontiguousarray(
        xT.reshape(DCH, P, 2, HS).transpose(1, 2, 0, 3)).astype(BF_NP)


def _np_reference(query, key, value, mask, Wq, bq, Wk, bk, Wv, bv):
    q = query @ Wq + bq
    k = key @ Wk + bk
    v = value @ Wv + bv
    scores = np.einsum("bqh,bkh->bqk", q, k) / np.sqrt(np.float32(H))
    scores = np.where(mask, scores, np.float32(-1e9))
    scores -= scores.max(axis=-1, keepdims=True)
    e = np.exp(scores)
    att = e / e.sum(axis=-1, keepdims=True)
    return np.einsum("bqk,bkh->bqh", att, v).astype(np.float32)


def kernel(query, key, value, mask, Wq, bq, Wk, bk, Wv, bv):
    global LAST_RESULTS
    query = np.asarray(query, dtype=np.float32)
    key = np.asarray(key, dtype=np.float32)
    value = np.asarray(value, dtype=np.float32)
    mask = np.asarray(mask).astype(bool)
    Wq = np.asarray(Wq, dtype=np.float32)
    Wk = np.asarray(Wk, dtype=np.float32)
    Wv = np.asarray(Wv, dtype=np.float32)
    bq = np.asarray(bq, dtype=np.float32)
    bk = np.asarray(bk, dtype=np.float32)
    bv = np.asarray(bv, dtype=np.float32)

    tril = np.tril(np.ones((S, S), dtype=bool))
    devpred = all(np.array_equal(mask[b], tril) for b in range(B))
    if not devpred:
        return _np_reference(query, key, value, mask, Wq, bq, Wk, bk,
                             Wv, bv)
    pairs, chunks_of, masked = CAUSAL_PAIRS, CAUSAL_CHUNKS, CAUSAL_MASKED
    key_v = ("causal_v2",)

    if key_v not in _PROGRAM_CACHE:
        _PROGRAM_CACHE[key_v] = _build_program(pairs, masked)
    nc = _PROGRAM_CACHE[key_v]

    def packw(w):
        return np.ascontiguousarray(
            w.reshape(DCH, P, H).transpose(1, 0, 2)).astype(BF_NP)

    wall_in = np.concatenate([packw(Wk), packw(Wv), packw(Wq)], axis=2)
    wall_in = np.ascontiguousarray(wall_in)
    ball_in = np.ascontiguousarray(
        np.stack([bk, bv, bq], axis=1).astype(np.float32))

    in_maps = []
    for c in range(NCORES):
        b, h = divmod(c, 2)
        chunks = chunks_of[h]
        q_rows = np.concatenate(
            [query[b, g * CHUNK:(g + 1) * CHUNK, :] for g in chunks], axis=0)
        qT = _pack_q(q_rows.T)
        kT = _pack_kv(key[b].T)
        vT = _pack_kv(value[b].T)
        im = {"qT": qT, "kT": kT, "vT": vT,
              "wall": wall_in, "ball": ball_in}
        thr = np.zeros((P, len(masked), 2), dtype=np.float32)
        pvec = np.arange(P, dtype=np.float32)
        for mi, (pr, jt) in enumerate(masked):
            for half in range(2):
                g = chunks[2 * pr + half]
                thr[:, mi, half] = jt * P + pvec - g * CHUNK
        im["thr"] = np.ascontiguousarray(thr)
        in_maps.append(im)

    results = run_bass_kernel_spmd(
        nc, in_maps, core_ids=list(range(NCORES)),
        trace=bool(os.environ.get("BASS_TRACE")),
    )
    LAST_RESULTS = results

    out = np.empty((B, S, H), dtype=np.float32)
    for c in range(NCORES):
        b, h = divmod(c, 2)
        chunks = chunks_of[h]
        o = results.results[c]["out"]          # [P, NQ//P, H] packed
        o = o.transpose(1, 0, 2).reshape(NQ, H)  # row t*128+p
        for s, g in enumerate(chunks):
            out[b, g * CHUNK:(g + 1) * CHUNK, :] = \
                o[s * CHUNK:(s + 1) * CHUNK]
    return out


# revision 10
# speedup vs baseline: 3.3682x; 3.3682x over previous
"""Distributed Trainium2 kernel for a single causal attention head.

Problem (hardcoded): B=4, S=2048, D_MODEL=1024, HEAD_DIM=64, fp32 inputs.
    q = query @ Wq + bq ; k = key @ Wk + bk ; v = value @ Wv + bv
    scores = q k^T / sqrt(H) ; masked softmax ; out = att @ v

Sharding (8 NeuronCores): core c = (b, h) with b = c//2, h = c%2.
Each core owns 4 query chunks of 256 rows of batch b and projects the
full k/v of its batch locally.  Chunks h=0 -> {0,3,4,7}, h=1 ->
{1,2,5,6} balance causal work under one SPMD program.  The program
works in two slot-PAIRS of 512 query rows; pair p computes j-tiles
[0, SHARED_p) at 512 width and [SHARED_p, SOLO_p) at 256 width (second
chunk only).  Causal: SHARED=(4,12), SOLO=(8,16) -- optimal given both
cores' chunk extents.  Per-core differences are pure data: gathered
query rows and device-generated predicate masks (from threshold
tables) that zero attention weights after exp.

v2 performance notes (vs v1 baseline, 60.7us):
  - All big inputs ride ONE HWDGE sync ring with 16KB-contiguous
    per-partition lines (kT/vT packed half-major [P,2,DCH,S/2]; qT one
    whole transfer) -> max descriptor size, ~358 GB/s instead of 313.
  - Ring order k0, q, k1, v0, v1: scores (k x q) start earliest.
  - ACT engine runs ONLY the 20 exp tiles (the mid-kernel pacer at
    ~680ns/512-wide tile).  Projection PSUM drains (+bias, fp32->bf16)
    moved to DVE tensor_scalar; predicate muls moved to GpSimd.
  - q projection col-paired like k/v (4096 PE cycles, was 8192).
  - Solo score tiles exp'd in PAIRS sharing one PSUM bank (one 512-wide
    exp for two 256-wide tiles) -> 20 exps instead of 24.
  - Output staged [P, NQ/P, H] and DMA'd packed (2KB contiguous per
    partition); host unpacks the (t p) -> rows permutation.
  - PE warm-up/bridge matmuls sized to keep HAM at 2.4 GHz through
    every data-wait gap (no >3us PE idle).
"""

import os

import numpy as np
import ml_dtypes

import concourse.bass as bass
import concourse.tile as tile
from concourse import bacc, mybir
from concourse.bass import ds
from concourse.bass_utils import run_bass_kernel_spmd
from concourse.masks import make_identity

B, S, D, H = 4, 2048, 1024, 64
P = 128
NCORES = 8
CHUNK = 256               # query rows per chunk/slot
NSLOTS = 4
NQ = NSLOTS * CHUNK       # 1024
JT = S // P               # 16 j-tiles of 128 keys
DCH = D // P              # 8 contraction chunks
HS = S // 2               # 1024 cols per half transfer
FP = mybir.dt.float32
BF = mybir.dt.bfloat16
BF_NP = ml_dtypes.bfloat16

# (shared extent, solo extent) per pair, in j-tiles
CAUSAL_PAIRS = ((4, 8), (12, 16))
CAUSAL_CHUNKS = {0: (0, 3, 4, 7), 1: (1, 2, 5, 6)}
# predicate entries: (pair, jt) for jt in [8p, 8p+8)
CAUSAL_MASKED = [(p, jt) for p in range(2) for jt in range(8 * p, 8 * p + 8)]

# warm-up / bridge matmul counts (N=512 each, ~216ns warm / ~427 cold)
WARM_A = 22       # kernel start -> k0 arrival (~6.3us to cover)
BRIDGE_KQ = 13    # kproj0 end -> q arrival (~3.9us)
BRIDGE_V1 = 6     # epi0 -> v1 arrival

LAST_RESULTS = None
_PROGRAM_CACHE = {}


def _build_program(pairs, masked_slots):
    """Build the SPMD Bass program (identical on all 8 cores)."""
    nc = bacc.Bacc("TRN2", target_bir_lowering=False, debug=False,
                   num_devices=NCORES)

    qT_d = nc.dram_tensor("qT", [P, DCH, NQ], BF, kind="ExternalInput").ap()
    kT_d = nc.dram_tensor("kT", [P, 2, DCH, HS], BF,
                          kind="ExternalInput").ap()
    vT_d = nc.dram_tensor("vT", [P, 2, DCH, HS], BF,
                          kind="ExternalInput").ap()
    wall_d = nc.dram_tensor("wall", [P, DCH, 3 * H], BF,
                            kind="ExternalInput").ap()
    ball_d = nc.dram_tensor("ball", [H, 3], FP, kind="ExternalInput").ap()
    nmask = len(masked_slots)
    thr_d = nc.dram_tensor("thr", [P, nmask, 2], FP,
                           kind="ExternalInput").ap()
    out_d = nc.dram_tensor("out", [P, NQ // P, H], FP,
                           kind="ExternalOutput").ap()

    with tile.TileContext(nc) as tc:
        with (
            tc.tile_pool(name="const", bufs=1) as const,
            tc.tile_pool(name="resident", bufs=1) as res,
            tc.tile_pool(name="attp", bufs=26) as attp,
            tc.tile_pool(name="outp", bufs=2) as outp,
            tc.tile_pool(name="pp", bufs=1, space="PSUM") as pp,
            tc.tile_pool(name="psc", bufs=3, space="PSUM") as psc,
            tc.tile_pool(name="pout", bufs=2, space="PSUM") as pout,
            tc.tile_pool(name="ptr", bufs=2, space="PSUM") as ptr,
        ):
            # ---- constants on the scalar HWDGE ring (thr first: the
            # predicate chain needs it earliest) ----
            thr_sb = const.tile([P, nmask, 2], FP, tag="thr")
            nc.scalar.dma_start(thr_sb, thr_d)
            wall_sb = const.tile([P, DCH, 3 * H], BF, tag="wall")
            nc.scalar.dma_start(wall_sb, wall_d)
            ball_sb = const.tile([H, 3], FP, tag="ball")
            nc.scalar.dma_start(ball_sb, ball_d)
            wk_sb = wall_sb[:, :, 0:H]
            wv_sb = wall_sb[:, :, H:2 * H]
            wq_sb = wall_sb[:, :, 2 * H:3 * H]
            bk_sb = ball_sb[:, 0:1]
            bv_sb = ball_sb[:, 1:2]
            bq_sb = ball_sb[:, 2:3]
            zeros_sb = const.tile([P, 512], BF, tag="zeros")
            nc.vector.memset(zeros_sb, 0.0)
            ident = const.tile([P, P], FP, tag="ident")
            make_identity(nc, ident)
            identb = const.tile([P, P], BF, tag="identb")
            make_identity(nc, identb)

            # ---- big inputs: ONE sync HWDGE ring, 16KB lines, strict
            # dependency order k0, q, k1, v0, v1 ----
            xk_sb = res.tile([P, 2, DCH, HS], BF, tag="xk")
            xv_sb = res.tile([P, 2, DCH, HS], BF, tag="xv")
            xq_sb = res.tile([P, DCH, NQ], BF, tag="xq")
            nc.sync.dma_start(xk_sb[:, 0], kT_d[:, 0])
            nc.sync.dma_start(xq_sb, qT_d)
            nc.sync.dma_start(xk_sb[:, 1], kT_d[:, 1])
            nc.sync.dma_start(xv_sb[:, 0], vT_d[:, 0])
            nc.sync.dma_start(xv_sb[:, 1], vT_d[:, 1])

            # predicate masks from per-core threshold tables:
            # pred[p, mi, half*256+f] = (f >= thr[p, mi, half])
            # Generated on DVE in two batches: pair0 entries before the
            # projection drains, pair1 entries after q_out (they are not
            # consumed until the pair1 exp stream, ~8 exps later).
            pred_sb = res.tile([P, nmask, 2 * CHUNK], BF, tag="pred")
            iota_sb = const.tile([P, CHUNK], FP, tag="iota")
            nc.gpsimd.iota(iota_sb, pattern=[[1, CHUNK]], base=0,
                           channel_multiplier=0,
                           allow_small_or_imprecise_dtypes=True)

            def gen_preds(mis):
                for mi in mis:
                    for half in range(2):
                        nc.vector.tensor_scalar(
                            pred_sb[:, mi, ds(half * CHUNK, CHUNK)],
                            iota_sb, thr_sb[:, mi, ds(half, 1)], None,
                            mybir.AluOpType.is_ge)

            gen_preds(range(8))

            # ---- PE warm-up: keep HAM at full clock until k0 lands
            pwarm = pp.tile([P, 512], FP, tag="pp", name="pwarm")

            def bridge(n):
                for _ in range(n):
                    nc.tensor.matmul(pwarm, lhsT=identb, rhs=zeros_sb,
                                     start=True, stop=True)

            bridge(WARM_A)

            # col-tiled projection pair: two M=64 matmuls run concurrently
            # in PE column groups 0 / 1 over two 512-wide input chunks.
            # x3: [P, DCH, 1024]; gc0: global output column of x3[..., 0]
            def proj_pair(w_sb, x3, gc0, out_fn, name):
                pj = pp.tile([P, 512], FP, tag="pp", name=name)
                for d in range(DCH):
                    nc.tensor.matmul(pj[0:H, :], lhsT=w_sb[:, d, :],
                                     rhs=x3[:, d, ds(0, 512)],
                                     start=(d == 0), stop=(d == DCH - 1),
                                     skip_group_check=True)
                    nc.tensor.matmul(pj[H:2 * H, :], lhsT=w_sb[:, d, :],
                                     rhs=x3[:, d, ds(512, 512)],
                                     start=(d == 0), stop=(d == DCH - 1),
                                     tile_position=(0, H),
                                     skip_group_check=True)
                out_fn(pj[0:H, :], gc0)
                out_fn(pj[H:2 * H, :], gc0 + 512)

            k_sb = res.tile([P, S], BF, tag="k")
            vT_sb = res.tile([P, S], BF, tag="vT")
            v_sb = res.tile([P, JT, H + 1], BF, tag="v")
            nc.vector.memset(v_sb[:, :, H:H + 1], 1.0)
            q_sb = res.tile([P, NQ], BF, tag="q")
            # zero q rows 64.. so the K=128 score contraction ignores
            # garbage k rows 64.. (zeroing one operand suffices)
            nc.vector.memset(q_sb[H:, :], 0.0)

            # PSUM drains on DVE (ACT is reserved for exp)
            def k_out(pj, c0):
                nc.vector.tensor_scalar(k_sb[:H, ds(c0, 512)], pj,
                                        bk_sb, None, mybir.AluOpType.add)

            def v_out(pj, c0):
                nc.vector.tensor_scalar(vT_sb[:H, ds(c0, 512)], pj,
                                        bv_sb, None, mybir.AluOpType.add)
                for jt in range(c0 // P, c0 // P + 4):
                    pvt = ptr.tile([P, P], BF, tag="tr", name="pvt")
                    nc.tensor.transpose(pvt, vT_sb[:, ds(jt * P, P)], identb)
                    nc.vector.tensor_copy(v_sb[:, jt, 0:H], pvt[:, :H])

            def q_out(pj, c0):
                nc.vector.tensor_scalar(q_sb[:H, ds(c0, 512)], pj,
                                        bq_sb, None, mybir.AluOpType.add)

            mask_idx = {sj: i for i, sj in enumerate(masked_slots)}
            W = 2 * CHUNK  # 512
            po_tiles = {}
            att_tiles = {}

            def emit_score_wide(pr, jt):
                c0 = pr * W
                ps = psc.tile([P, W], FP, tag="sc", name="ps")
                nc.tensor.matmul(ps, lhsT=k_sb[:, ds(jt * P, P)],
                                 rhs=q_sb[:, ds(c0, W)],
                                 start=True, stop=True)
                att = attp.tile([P, W], BF, tag="att", name="att")
                nc.scalar.activation(att, ps,
                                     mybir.ActivationFunctionType.Exp,
                                     scale=0.125)
                mi = mask_idx.get((pr, jt))
                if mi is not None:
                    nc.vector.tensor_mul(att, att, pred_sb[:, mi, :])
                att_tiles[(pr, jt)] = (att, c0, W)

            def emit_score_solo_pair(pr, jt0):
                # two 256-wide solo tiles (jt0, jt0+1) share one PSUM
                # bank and one 512-wide exp
                c0 = pr * W + CHUNK
                ps = psc.tile([P, W], FP, tag="sc", name="ps")
                for i in range(2):
                    nc.tensor.matmul(ps[:, ds(i * CHUNK, CHUNK)],
                                     lhsT=k_sb[:, ds((jt0 + i) * P, P)],
                                     rhs=q_sb[:, ds(c0, CHUNK)],
                                     start=True, stop=True,
                                     skip_group_check=True)
                att = attp.tile([P, W], BF, tag="att", name="att")
                nc.scalar.activation(att, ps,
                                     mybir.ActivationFunctionType.Exp,
                                     scale=0.125)
                for i in range(2):
                    mi = mask_idx.get((pr, jt0 + i))
                    if mi is not None:
                        nc.vector.tensor_mul(
                            att[:, ds(i * CHUNK, CHUNK)],
                            att[:, ds(i * CHUNK, CHUNK)],
                            pred_sb[:, mi, ds(CHUNK, CHUNK)])
                    att_tiles[(pr, jt0 + i)] = (att, c0, CHUNK, i * CHUNK)

            def emit_av(pr, jt):
                solo = pairs[pr][1]
                if pr not in po_tiles:
                    po_tiles[pr] = pout.tile([H + 1, W], FP, tag="po",
                                             name=f"po{pr}")
                rec = att_tiles.pop((pr, jt))
                att, c0, n = rec[0], rec[1], rec[2]
                src = att if len(rec) == 3 else att[:, ds(rec[3], n)]
                nc.tensor.matmul(po_tiles[pr][:, ds(c0 - pr * W, n)],
                                 lhsT=v_sb[:, jt, :], rhs=src,
                                 start=(jt == 0), stop=(jt == solo - 1),
                                 skip_group_check=True)

            out_stage = res.tile([P, NQ // P, H], FP, tag="ostage")

            def epilogue(pr):
                po = po_tiles[pr]
                oT_sb = outp.tile([P, W], FP, tag="oT")
                nc.vector.tensor_copy(oT_sb[:H + 1, :], po)
                for t in range(W // P):
                    pt = ptr.tile([P, P], FP, tag="tr")
                    nc.tensor.transpose(pt, oT_sb[:, ds(t * P, P)], ident)
                    recip = outp.tile([P, 1], FP, tag="recip")
                    nc.vector.reciprocal(recip, pt[:, H:H + 1])
                    nc.vector.tensor_scalar_mul(
                        out_stage[:, pr * (W // P) + t, :], pt[:, :H], recip)

            # ---- decoupled emission schedule (engines are in-order;
            # each group sits at its data-arrival position) ----
            sh0, so0 = pairs[0]
            sh1, so1 = pairs[1]

            proj_pair(wk_sb, xk_sb[:, 0], 0, k_out, "pk0")
            bridge(BRIDGE_KQ)
            proj_pair(wq_sb, xq_sb, 0, q_out, "pq")
            gen_preds(range(8, 16))
            for jt in range(sh0):
                emit_score_wide(0, jt)
            for jt0 in range(sh0, so0, 2):
                emit_score_solo_pair(0, jt0)
            for jt in range(8):
                emit_score_wide(1, jt)
            proj_pair(wk_sb, xk_sb[:, 1], HS, k_out, "pk1")
            for jt in range(8, sh1):
                emit_score_wide(1, jt)
            for jt0 in range(sh1, so1, 2):
                emit_score_solo_pair(1, jt0)
            proj_pair(wv_sb, xv_sb[:, 0], 0, v_out, "pv0")
            for jt in range(so0):
                emit_av(0, jt)
            for jt in range(8):
                emit_av(1, jt)
            epilogue(0)
            bridge(BRIDGE_V1)
            proj_pair(wv_sb, xv_sb[:, 1], HS, v_out, "pv1")
            for jt in range(8, so1):
                emit_av(1, jt)
            epilogue(1)
            nc.gpsimd.dma_start(out_d, out_stage)

    nc.compile()
    return nc


def _slot_extents(pairs):
    return (pairs[0][0], pairs[0][1], pairs[1][0], pairs[1][1])


def _mask_fits_causal_variant(mask):
    extents = _slot_extents(CAUSAL_PAIRS)
    for h, chunks in CAUSAL_CHUNKS.items():
        for s, g in enumerate(chunks):
            rows = slice(g * CHUNK, (g + 1) * CHUNK)
            bound = extents[s] * P
            lo = (8 * (s // 2)) * P
            if bound < S and mask[:, rows, bound:].any():
                return False
            if lo > 0 and not mask[:, rows, :lo].all():
                return False
    return True


def _pack_q(xT):
    """[D, NQ] -> [128, D/128, NQ]."""
    d, s = xT.shape
    return np.ascontiguousarray(
        xT.reshape(DCH, P, s).transpose(1, 0, 2)).astype(BF_NP)


def _pack_kv(xT):
    """[D, S] -> [128, 2, D/128, S/2]: 16KB contiguous per partition
    per half transfer."""
    d, s = xT.shape
    return np.ascontiguousarray(
        xT.reshape(DCH, P, 2, HS).transpose(1, 2, 0, 3)).astype(BF_NP)


def _np_reference(query, key, value, mask, Wq, bq, Wk, bk, Wv, bv):
    q = query @ Wq + bq
    k = key @ Wk + bk
    v = value @ Wv + bv
    scores = np.einsum("bqh,bkh->bqk", q, k) / np.sqrt(np.float32(H))
    scores = np.where(mask, scores, np.float32(-1e9))
    scores -= scores.max(axis=-1, keepdims=True)
    e = np.exp(scores)
    att = e / e.sum(axis=-1, keepdims=True)
    return np.einsum("bqk,bkh->bqh", att, v).astype(np.float32)


def kernel(query, key, value, mask, Wq, bq, Wk, bk, Wv, bv):
    global LAST_RESULTS
    query = np.asarray(query, dtype=np.float32)
    key = np.asarray(key, dtype=np.float32)
    value = np.asarray(value, dtype=np.float32)
    mask = np.asarray(mask).astype(bool)
    Wq = np.asarray(Wq, dtype=np.float32)
    Wk = np.asarray(Wk, dtype=np.float32)
    Wv = np.asarray(Wv, dtype=np.float32)
    bq = np.asarray(bq, dtype=np.float32)
    bk = np.asarray(bk, dtype=np.float32)
    bv = np.asarray(bv, dtype=np.float32)

    tril = np.tril(np.ones((S, S), dtype=bool))
    devpred = all(np.array_equal(mask[b], tril) for b in range(B))
    if not devpred:
        return _np_reference(query, key, value, mask, Wq, bq, Wk, bk,
                             Wv, bv)
    pairs, chunks_of, masked = CAUSAL_PAIRS, CAUSAL_CHUNKS, CAUSAL_MASKED
    key_v = ("causal_v2",)

    if key_v not in _PROGRAM_CACHE:
        _PROGRAM_CACHE[key_v] = _build_program(pairs, masked)
    nc = _PROGRAM_CACHE[key_v]

    def packw(w):
        return np.ascontiguousarray(
            w.reshape(DCH, P, H).transpose(1, 0, 2)).astype(BF_NP)

    wall_in = np.concatenate([packw(Wk), packw(Wv), packw(Wq)], axis=2)
    wall_in = np.ascontiguousarray(wall_in)
    ball_in = np.ascontiguousarray(
        np.stack([bk, bv, bq], axis=1).astype(np.float32))

    in_maps = []
    for c in range(NCORES):
        b, h = divmod(c, 2)
        chunks = chunks_of[h]
        q_rows = np.concatenate(
            [query[b, g * CHUNK:(g + 1) * CHUNK, :] for g in chunks], axis=0)
        qT = _pack_q(q_rows.T)
        kT = _pack_kv(key[b].T)
        vT = _pack_kv(value[b].T)
        im = {"qT": qT, "kT": kT, "vT": vT,
              "wall": wall_in, "ball": ball_in}
        thr = np.zeros((P, len(masked), 2), dtype=np.float32)
        pvec = np.arange(P, dtype=np.float32)
        for mi, (pr, jt) in enumerate(masked):
            for half in range(2):
                g = chunks[2 * pr + half]
                thr[:, mi, half] = jt * P + pvec - g * CHUNK
        im["thr"] = np.ascontiguousarray(thr)
        in_maps.append(im)

    results = run_bass_kernel_spmd(
        nc, in_maps, core_ids=list(range(NCORES)),
        trace=bool(os.environ.get("BASS_TRACE")),
    )
    LAST_RESULTS = results

    out = np.empty((B, S, H), dtype=np.float32)
    for c in range(NCORES):
        b, h = divmod(c, 2)
        chunks = chunks_of[h]
        o = results.results[c]["out"]          # [P, NQ//P, H] packed
        o = o.transpose(1, 0, 2).reshape(NQ, H)  # row t*128+p
        for s, g in enumerate(chunks):
            out[b, g * CHUNK:(g + 1) * CHUNK, :] = \
                o[s * CHUNK:(s + 1) * CHUNK]
    return out
